# revision 11
# baseline (speedup 1.0000x reference)
"""Self-contained Trainium2 Bass kernel: fused multi-head self-attention + LayerNorm.

Problem: B=4, S=2048, D=768, H=12 (head_dim 64), fp32 reference.

Sharding (no collectives): 8 cores = (batch b, query-half hf).  Each core
computes K/V for its full batch sequence (S=2048) and the attention output,
projection, residual and LayerNorm for its own 1024 query rows.  The context
is host-rolled per core so the core's query rows are always rows 0:1024
(softmax/attention are permutation-invariant over the key axis).

Device pipeline per core:
  1. x [2048,768] -> SBUF, PE-transpose -> xT [768,2048] (fp32)
  2. QKV: fp32r matmuls  qT=[wq.T x.T] (bf16 out), kT (bf16), V normal (bf16)
     (1/sqrt(64) folded into wq on host)
  3. per head pair: scores^T = kT.T@qT (row-tiled, bf16), exp on ACT with
     per-ktok mask bias (folds the pad mask), P^T bf16; attnV col-tiled
     (V as stationary) accumulated over ktok in PSUM; softmax denominators
     via ones-vector matmuls; normalization via broadcast reciprocal.
  4. proj (bf16) + b_proj + residual + LayerNorm (fp32 stats), DMA out.
"""

import os
from contextlib import ExitStack

import numpy as np

import concourse.bass as bass
import concourse.bacc as bacc
import concourse.tile as tile
from concourse import mybir
from concourse.masks import make_identity
from concourse.bass_utils import run_bass_kernel_spmd

# ---- problem constants (hardcoded per harness contract) ----
B, S, D, H = 4, 2048, 768, 12
HD = 64
P = 128
NCORES = 8
SQ = S // 2  # query rows per core

F32 = mybir.dt.float32
F32R = mybir.dt.float32r
BF16 = mybir.dt.bfloat16
AF = mybir.ActivationFunctionType
ALU = mybir.AluOpType
EPS = 1e-5

TRACE = bool(int(os.environ.get("KERNEL_TRACE", "0")))
LAST_RESULTS = None


def _r(ap):
    """bitcast an fp32 AP to fp32r for full-rate PE matmuls"""
    return ap.bitcast(F32R)


def declare_io(nc, S_, SQ_, D_):
    io = {}
    io["ctx"] = nc.dram_tensor("ctx", [S_, D_], F32, kind="ExternalInput")
    io["wq"] = nc.dram_tensor("wq", [D_, D_], BF16, kind="ExternalInput")
    io["wk"] = nc.dram_tensor("wk", [D_, D_], BF16, kind="ExternalInput")
    io["wv"] = nc.dram_tensor("wv", [D_, D_], BF16, kind="ExternalInput")
    io["wp"] = nc.dram_tensor("wp", [D_, D_], BF16, kind="ExternalInput")
    io["bproj"] = nc.dram_tensor("bproj", [D_], F32, kind="ExternalInput")
    io["gamma"] = nc.dram_tensor("gamma", [D_], F32, kind="ExternalInput")
    io["beta"] = nc.dram_tensor("beta", [D_], F32, kind="ExternalInput")
    io["mbias"] = nc.dram_tensor("mbias", [P, S_ // P], F32, kind="ExternalInput")
    io["out"] = nc.dram_tensor("out", [SQ_, D_], F32, kind="ExternalOutput")
    return io


def emit_mhsa(ctx: ExitStack, tc: tile.TileContext, io, S_, SQ_, D_, H_):
    nc = tc.nc
    KT = S_ // P        # key-token tiles
    DT = D_ // P        # feature tiles (also head pairs)
    PAIRS = H_ // 2
    assert PAIRS == DT
    QB = min(512, SQ_)  # query block (matmul N)
    NQB = SQ_ // QB
    QTT = SQ_ // P      # query token tiles (proj phase)

    ctx_r = io["ctx"][:, :].rearrange("(t p) d -> p t d", p=P)
    out_r = io["out"][:, :].rearrange("(t p) d -> p t d", p=P)

    # ---------------- constants ----------------
    const = ctx.enter_context(tc.tile_pool(name="const", bufs=1))
    ident = const.tile([P, P], BF16)
    make_identity(nc, ident)
    mb = const.tile([P, KT], F32)
    nc.gpsimd.dma_start(out=mb, in_=io["mbias"][:, :])
    eps_t = const.tile([P, 1], F32)
    nc.vector.memset(eps_t, EPS)
    ones_t = const.tile([P, 1], BF16)
    nc.vector.memset(ones_t, 1.0)
    bproj_bc = const.tile([P, D_], F32)
    nc.gpsimd.dma_start(out=bproj_bc, in_=io["bproj"][:].partition_broadcast(P))
    gamma_bc = const.tile([P, D_], F32)
    nc.gpsimd.dma_start(out=gamma_bc, in_=io["gamma"][:].partition_broadcast(P))
    beta_bc = const.tile([P, D_], F32)
    nc.gpsimd.dma_start(out=beta_bc, in_=io["beta"][:].partition_broadcast(P))

    # ---------------- persistent big tiles ----------------
    big = ctx.enter_context(tc.tile_pool(name="big", bufs=1))
    qT = big.tile([P, DT, SQ_], BF16, tag="qT")     # [pair-feats, pair, qtok]
    kTt = big.tile([P, DT, S_], BF16, tag="kT")     # [pair-feats, pair, ktok]
    V = big.tile([P, KT, D_], BF16, tag="V")        # [tok-in-tile, ktile, feat]
    attnT = big.tile([P, DT, SQ_], BF16, tag="attnT")

    # ---------------- phase 1: load + transpose x ----------------
    with tc.tile_pool(name="xt_pool", bufs=1) as xt_pool, \
         tc.tile_pool(name="xn_pool", bufs=3) as xn_pool, \
         tc.tile_pool(name="tr_ps", bufs=2, space="PSUM") as tr_ps:
        xT = xt_pool.tile([P, DT, S_], BF16, tag="xT")
        for t in range(KT):
            xn = xn_pool.tile([P, D_], F32, tag="xn")
            nc.gpsimd.dma_start(out=xn, in_=ctx_r[:, t, :])
            xb = xn_pool.tile([P, D_], BF16, tag="xb")
            nc.vector.tensor_copy(out=xb, in_=xn)
            for d in range(DT):
                ps = tr_ps.tile([P, P], BF16, tag="trps")
                nc.tensor.transpose(ps, xb[:, d * P:(d + 1) * P], ident)
                nc.vector.tensor_copy(out=xT[:, d, t * P:(t + 1) * P], in_=ps)

        # ---------------- phase 2: QKV projections (fp32r) ----------------
        with tc.tile_pool(name="wpool", bufs=6) as wpool, \
             tc.tile_pool(name="qkv_ps", bufs=2, space="PSUM") as qkv_ps:
            # q and k transposed outputs: [feat, tok]
            for wio, dst, ntok in ((io["wq"], qT, SQ_), (io["wk"], kTt, S_)):
                nblk = min(512, ntok)
                wsb = []
                for din in range(DT):
                    w_t = wpool.tile([P, D_], BF16, tag="wsb")
                    nc.gpsimd.dma_start(out=w_t, in_=wio[din * P:(din + 1) * P, :])
                    wsb.append(w_t)
                for m in range(DT):
                    for nb in range(ntok // nblk):
                        pq = qkv_ps.tile([P, 512], F32, tag="qkvps")
                        for din in range(DT):
                            nc.tensor.matmul(
                                pq[:, 0:nblk],
                                lhsT=wsb[din][:, m * P:(m + 1) * P],
                                rhs=xT[:, din, nb * nblk:(nb + 1) * nblk],
                                start=(din == 0),
                                stop=(din == DT - 1),
                            )
                        nc.vector.tensor_copy(
                            out=dst[:, m, nb * nblk:(nb + 1) * nblk],
                            in_=pq[:, 0:nblk])
            # V in normal orientation: [tok, feat]
            wsb = []
            for din in range(DT):
                w_t = wpool.tile([P, D_], BF16, tag="wsb")
                nc.gpsimd.dma_start(out=w_t, in_=io["wv"][din * P:(din + 1) * P, :])
                wsb.append(w_t)
            for t in range(KT):
                for c0, cw in ((0, 512), (512, 256)):
                    pv = qkv_ps.tile([P, 512], F32, tag="qkvps")
                    for din in range(DT):
                        nc.tensor.matmul(
                            pv[:, 0:cw],
                            lhsT=xT[:, din, t * P:(t + 1) * P],
                            rhs=wsb[din][:, c0:c0 + cw],
                            start=(din == 0),
                            stop=(din == DT - 1),
                        )
                    nc.vector.tensor_copy(out=V[:, t, c0:c0 + cw], in_=pv[:, 0:cw])

    # ---------------- phase 3: attention (bf16 matmuls) ----------------
    with tc.tile_pool(name="s_ps", bufs=2, space="PSUM") as s_ps, \
         tc.tile_pool(name="av_ps", bufs=2, space="PSUM") as av_ps, \
         tc.tile_pool(name="den_ps", bufs=1, space="PSUM") as den_ps, \
         tc.tile_pool(name="pt_pool", bufs=3) as pt_pool, \
         tc.tile_pool(name="dr_pool", bufs=2, space="DRAM") as dr_pool, \
         tc.tile_pool(name="r_pool", bufs=2) as r_pool:
        for p in range(PAIRS):
            av = [av_ps.tile([P, QB], F32, tag="av", name=f"av_{p}_{i}")
                  for i in range(NQB)]
            den = den_ps.tile([P, 512], F32, tag="den")
            for kt in range(KT):
                for qbi in range(NQB):
                    sh = s_ps.tile([P, 2 * QB], F32, tag="sh")
                    # transposed scores, head pair row-tiled on the PE
                    nc.tensor.matmul(
                        sh[:, 0:QB],
                        lhsT=kTt[0:HD, p, kt * P:(kt + 1) * P],
                        rhs=qT[0:HD, p, qbi * QB:(qbi + 1) * QB],
                        start=True, stop=True, tile_position=(0, 0),
                    )
                    nc.tensor.matmul(
                        sh[:, QB:2 * QB],
                        lhsT=kTt[HD:P, p, kt * P:(kt + 1) * P],
                        rhs=qT[HD:P, p, qbi * QB:(qbi + 1) * QB],
                        start=True, stop=True, tile_position=(64, 0),
                    )
                    # exp(scores + mask_bias[ktok]) -> P^T (bf16)
                    pt = pt_pool.tile([P, 2 * QB], BF16, tag="pt")
                    nc.scalar.activation(
                        out=pt, in_=sh, func=AF.Exp,
                        bias=mb[:, kt:kt + 1], scale=1.0)
                    # attnV: V stationary, col-tiled head pair shares the bank
                    # (independent accumulation groups share a PSUM bank at
                    # disjoint partition ranges; the sim's group check is
                    # bank-global, so it must be skipped — pending-zero
                    # tracking itself is per-partition-row.)
                    nc.tensor.matmul(
                        av[qbi][0:HD, :],
                        lhsT=V[:, kt, p * P:p * P + HD],
                        rhs=pt[:, 0:QB],
                        start=(kt == 0), stop=(kt == KT - 1),
                        tile_position=(0, 0), skip_group_check=True,
                    )
                    nc.tensor.matmul(
                        av[qbi][HD:P, :],
                        lhsT=V[:, kt, p * P + HD:(p + 1) * P],
                        rhs=pt[:, QB:2 * QB],
                        start=(kt == 0), stop=(kt == KT - 1),
                        tile_position=(0, 64), skip_group_check=True,
                    )
                    # softmax denominators: ones . P^T, col-tiled M=1
                    for hh in range(2):
                        rr = (qbi * 2 + hh) * 32
                        nc.tensor.matmul(
                            den[rr:rr + 1, 0:QB],
                            lhsT=ones_t,
                            rhs=pt[:, hh * QB:(hh + 1) * QB],
                            start=(kt == 0), stop=(kt == KT - 1),
                            tile_position=(0, rr), skip_group_check=True,
                        )
            # normalization: R = 1/den broadcast across partitions.
            # SBUF APs can't broadcast (partition step 0), so bounce the
            # reciprocal rows through DRAM and broadcast-read them back.
            den_sb = r_pool.tile([P, 512], F32, tag="den_sb")
            den_dr = dr_pool.tile([2 * NQB, QB], F32, tag="den_dr")
            R = r_pool.tile([P, SQ_], F32, tag="R")
            for qbi in range(NQB):
                for hh in range(2):
                    rr = (qbi * 2 + hh) * 32
                    nc.vector.reciprocal(
                        out=den_sb[rr:rr + 1, 0:QB], in_=den[rr:rr + 1, 0:QB])
                    nc.gpsimd.dma_start(
                        out=den_dr[qbi * 2 + hh:qbi * 2 + hh + 1, :],
                        in_=den_sb[rr:rr + 1, 0:QB])
                    nc.gpsimd.dma_start(
                        out=R[hh * HD:(hh + 1) * HD, qbi * QB:(qbi + 1) * QB],
                        in_=den_dr[qbi * 2 + hh, :].partition_broadcast(HD),
                    )
            for qbi in range(NQB):
                nc.vector.tensor_mul(
                    out=attnT[:, p, qbi * QB:(qbi + 1) * QB],
                    in0=av[qbi],
                    in1=R[:, qbi * QB:(qbi + 1) * QB],
                )

    # ---------------- phase 4: projection + residual + LayerNorm ----------------
    with tc.tile_pool(name="wp_pool", bufs=6) as wp_pool, \
         tc.tile_pool(name="proj_ps", bufs=2, space="PSUM") as proj_ps, \
         tc.tile_pool(name="res_pool", bufs=2) as res_pool, \
         tc.tile_pool(name="y_pool", bufs=2) as y_pool, \
         tc.tile_pool(name="st_pool", bufs=4) as st_pool:
        wpsb = []
        for din in range(DT):
            w_t = wp_pool.tile([P, D_], BF16, tag="wpsb")
            nc.gpsimd.dma_start(out=w_t, in_=io["wp"][din * P:(din + 1) * P, :])
            wpsb.append(w_t)
        for t in range(QTT):
            x_res = res_pool.tile([P, D_], F32, tag="xres")
            nc.gpsimd.dma_start(out=x_res, in_=ctx_r[:, t, :])
            y = y_pool.tile([P, D_], F32, tag="y")
            for c0, cw in ((0, 512), (512, 256)):
                pp = proj_ps.tile([P, 512], F32, tag="projps")
                for din in range(DT):
                    nc.tensor.matmul(
                        pp[:, 0:cw],
                        lhsT=attnT[:, din, t * P:(t + 1) * P],
                        rhs=wpsb[din][:, c0:c0 + cw],
                        start=(din == 0),
                        stop=(din == DT - 1),
                    )
                nc.vector.tensor_add(
                    out=y[:, c0:c0 + cw], in0=pp[:, 0:cw], in1=x_res[:, c0:c0 + cw])
            nc.vector.tensor_add(out=y, in0=y, in1=bproj_bc)
            # LayerNorm over D
            nsub = D_ // 256
            stats = st_pool.tile([P, nsub, nc.vector.BN_STATS_DIM], F32, tag="stats")
            for g in range(nsub):
                nc.vector.bn_stats(out=stats[:, g, :], in_=y[:, g * 256:(g + 1) * 256])
            mv = st_pool.tile([P, 2], F32, tag="mv")
            nc.vector.bn_aggr(out=mv, in_=stats)
            sd = st_pool.tile([P, 1], F32, tag="sd")
            nc.scalar.activation(
                out=sd, in_=mv[:, 1:2], func=AF.Sqrt, bias=eps_t, scale=1.0)
            nc.vector.reciprocal(out=sd, in_=sd)
            nc.vector.tensor_scalar(
                out=y, in0=y, scalar1=mv[:, 0:1], scalar2=sd,
                op0=ALU.subtract, op1=ALU.mult)
            nc.vector.tensor_mul(out=y, in0=y, in1=gamma_bc)
            nc.vector.tensor_add(out=y, in0=y, in1=beta_bc)
            nc.gpsimd.dma_start(out=out_r[:, t, :], in_=y)


def build_program(S_=S, SQ_=SQ, D_=D, H_=H):
    nc = bacc.Bacc("TRN2")
    io = declare_io(nc, S_, SQ_, D_)
    with tile.TileContext(nc) as tc:
        with ExitStack() as ctx:
            emit_mhsa(ctx, tc, io, S_, SQ_, D_, H_)
    nc.compile()
    return nc, io


def prep_inputs(context, pad_mask, w_qkv, w_proj, b_proj, gamma, beta,
                S_=S, SQ_=SQ, D_=D, ncores=NCORES):
    import ml_dtypes
    context = np.asarray(context, dtype=np.float32)
    pad_mask = np.asarray(pad_mask, dtype=np.float32)
    w_qkv = np.asarray(w_qkv, dtype=np.float32)
    wq = (np.ascontiguousarray(w_qkv[:, 0:D_])
          * np.float32(1.0 / np.sqrt(HD))).astype(ml_dtypes.bfloat16)
    wk = np.ascontiguousarray(w_qkv[:, D_:2 * D_]).astype(ml_dtypes.bfloat16)
    wv = np.ascontiguousarray(w_qkv[:, 2 * D_:3 * D_]).astype(ml_dtypes.bfloat16)
    wp = np.asarray(w_proj, dtype=np.float32).astype(ml_dtypes.bfloat16)
    bp = np.asarray(b_proj, dtype=np.float32)
    ga = np.asarray(gamma, dtype=np.float32)
    be = np.asarray(beta, dtype=np.float32)
    mbias = (pad_mask - 1.0) * np.float32(1e10)  # [B, S]
    in_maps = []
    for c in range(ncores):
        b, hf = c // 2, c % 2
        ctx_c = np.ascontiguousarray(np.roll(context[b], -hf * SQ_, axis=0))
        mb_c = np.ascontiguousarray(
            np.roll(mbias[b], -hf * SQ_).reshape(S_ // P, P).T).astype(np.float32)
        in_maps.append({
            "ctx": ctx_c, "wq": wq, "wk": wk, "wv": wv, "wp": wp,
            "bproj": bp, "gamma": ga, "beta": be, "mbias": mb_c,
        })
    return in_maps


def kernel(context, pad_mask, w_qkv, w_proj, b_proj, gamma, beta):
    global LAST_RESULTS
    nc, _io = build_program()
    in_maps = prep_inputs(context, pad_mask, w_qkv, w_proj, b_proj, gamma, beta)
    res = run_bass_kernel_spmd(nc, in_maps, core_ids=list(range(NCORES)),
                               trace=TRACE)
    LAST_RESULTS = res
    out = np.empty((B, S, D), np.float32)
    for c in range(NCORES):
        b, hf = c // 2, c % 2
        out[b, hf * SQ:(hf + 1) * SQ] = res.results[c]["out"]
    return out
